# revision 12
# baseline (speedup 1.0000x reference)
"""Cross-attention Trainium2 kernel (B=8, N=2048, C=768, head=1).

reference:
  q = q_x @ Wq.T ; k = k_x @ Wk.T
  S = (q @ k.T) / 768 ; P = softmax(S, -1) ; out = P @ v_x

Strategy (per core, data-parallel over batch), fp8 DoubleRow everywhere:
  M16 = 16 * Wq.T @ Wk  (host, fp8)
  qT/kT via XBAR DMA-transpose (bf16, per-block jobs) -> cast to fp8
  t16T[c2, n] = sum_c1 M16[c1,c2] * qT[c1,n]     (fp8 DR, psum f32 -> fp8)
  S16[m, n]  = sum_c2 kT[c2,m] * t16T[c2,n]      (fp8 DR)  == 16 * S_true
  PT16 = exp(S16/(768*16) + ln16) = 16*exp(a)    (ACT, bf16)
  E16  = PT16 - 16 = 16*(exp(a)-1)               (DVE, fp8; |E16| ~ 0.6)
  O[n, 0:784] = 16*colsum([v|1]) + sum_m E16[m,n] * vb8[m, 0:784]
    colsum injected via identity f32r matmul (start of psum group),
    E-part via fp8 DR. col 768 = 16*Z (softmax denominator * 16).
  out[n, c] = O[n, c] / O[n, 768]    (bf16 out, host upcasts)

Mean-subtraction is load-bearing: attention here is near-uniform
(a ~ N(0, 0.036)), so out ~= colmean(v) and coherent fp8 noise on P or v
does not average down (naive fp8 P@v measures 3.6e-2 rel err, over the
2e-2 gate). Quantizing only the deviation E = P-1 scales that noise by
~0.036: measured 3.2e-3 end to end in numpy.

Engine split: PE does t/S/PV DR matmuls + colsum (woven after S(0), when
the v tiles have landed behind the XBAR transpose storm). ACT does exp
and the final normalize. DVE does q/t casts, E16, reciprocals. GpSimd
does the k/v/csum casts so a late v DMA can never stall the E16 chain.
PE order S(nb+1) before PV(nb) keeps exp/E16 latency off the PE path.
"""

import sys

sys.path.insert(0, "/opt/trn_rl_repo")

from contextlib import ExitStack

import numpy as np

import concourse.bass as bass
import concourse.mybir as mybir
import concourse.tile as tile
from concourse import bacc
from concourse.masks import make_identity

F32 = mybir.dt.float32
F32R = mybir.dt.float32r
BF16 = mybir.dt.bfloat16
F8 = mybir.dt.float8e4
DR = mybir.MatmulPerfMode.DoubleRow

B = 8
N = 2048
C = 768
P = 128
CC = C // P          # 6 chunks of the channel dim
NN = N // P          # 16 chunks of the sequence dim
BLK = 512            # free-dim block (PSUM bank = 512 f32)
NB = N // BLK        # 4 sequence blocks
VW = C + 16          # padded v width: [v | 1 | 0*15], stride mult of 16
SCALE16 = 1.0 / float(C * 16)
LN16 = float(np.log(16.0))
EXP = mybir.ActivationFunctionType.Exp
COPY = mybir.ActivationFunctionType.Copy
SUB = mybir.AluOpType.subtract


def build_kernel():
    nc = bacc.Bacc("TRN2", target_bir_lowering=False, debug=False, num_devices=B)
    q_x = nc.declare_dram_parameter("q_x", [N, C], BF16, isOutput=False)
    k_x = nc.declare_dram_parameter("k_x", [N, C], BF16, isOutput=False)
    v_x = nc.declare_dram_parameter("v_x", [N, C], BF16, isOutput=False)
    Mw = nc.declare_dram_parameter("Mw", [C, C], F8, isOutput=False)
    out = nc.declare_dram_parameter("out", [N, C], BF16, isOutput=True)

    with tile.TileContext(nc) as tc, ExitStack() as ctx:
        persist = ctx.enter_context(tc.tile_pool(name="persist", bufs=1))
        qT8 = persist.tile([P, CC, N], F8)      # q_x.T fp8 [c1, n]
        kT8 = persist.tile([P, CC, N], F8)      # k_x.T fp8 [c2, m]
        tT8 = persist.tile([P, CC, N], F8)      # t16.T fp8 [c2, n]
        sbM8 = persist.tile([P, CC, C], F8)     # M16 [c1, c2]
        vb8 = persist.tile([P, NN, VW], F8)     # [v | 1 | 0...] fp8
        csum = persist.tile([P, VW], F32R)      # 16*colsum [v | 1 | 0], all rows
        identf = persist.tile([P, P], F32)
        ident = persist.tile([P, P], F32R)
        all16 = persist.tile([P, P], BF16)
        ln16 = persist.tile([P, 1], F32)
        onepad = persist.tile([P, NN, VW - C], F32)
        cspad = persist.tile([P, VW - C], F32)
        make_identity(nc, identf)
        nc.vector.tensor_copy(out=ident, in_=identf)
        nc.vector.memset(all16, 16.0)
        nc.vector.memset(ln16, LN16)
        nc.vector.memset(onepad[:, :, 0:1], 1.0)
        nc.vector.memset(onepad[:, :, 1:], 0.0)
        nc.vector.tensor_copy(out=vb8[:, :, C:VW], in_=onepad)
        nc.vector.memset(cspad[:, 0:1], 32768.0)
        nc.vector.memset(cspad[:, 1:], 0.0)
        nc.vector.tensor_copy(out=csum[:, C:VW], in_=cspad)

        bfpool = ctx.enter_context(tc.tile_pool(name="bfstage", bufs=1))
        qTb = [bfpool.tile([P, CC, BLK], BF16, name=f"qTb{nb}") for nb in range(NB)]
        kTb = [bfpool.tile([P, CC, BLK], BF16, name=f"kTb{nb}") for nb in range(NB)]
        vstage = [bfpool.tile([P, C], BF16, name=f"v{mc}") for mc in range(NN)]

        # ---------------- prologue DMAs ----------------
        # sync ring: M16 chunks then q transposes (t-matmul critical path);
        # scalar ring: k transposes (needed from S(0)); gpsimd ring: v tiles.
        for c1c in range(CC):
            nc.sync.dma_start(
                out=sbM8[:, c1c, :], in_=Mw[c1c * P : (c1c + 1) * P, :]
            )
        for nb in range(NB):
            nc.sync.dma_start_transpose(
                out=qTb[nb], in_=q_x[nb * BLK : (nb + 1) * BLK, :]
            )
            nc.scalar.dma_start_transpose(
                out=kTb[nb], in_=k_x[nb * BLK : (nb + 1) * BLK, :]
            )
        for mc in range(NN):
            nc.gpsimd.dma_start(out=vstage[mc], in_=v_x[mc * P : (mc + 1) * P, :])

        # gpsimd compute: k casts early (S(0) needs kT8 right after t-phase),
        # then v casts + csum staging off the DVE queue.
        for nb in range(NB):
            nc.gpsimd.tensor_copy(
                out=kT8[:, :, nb * BLK : (nb + 1) * BLK], in_=kTb[nb]
            )
        for mc in range(NN):
            nc.gpsimd.tensor_copy(out=vb8[:, mc, 0:C], in_=vstage[mc])

        # ---------------- PE warmup (HAM un-throttle) ----------------
        with (
            tc.tile_pool(name="warm", bufs=1) as warm_pool,
            tc.tile_pool(name="warm_psum", bufs=1, space="PSUM") as warm_psum,
        ):
            wl = warm_pool.tile([P, P], BF16)
            wr = warm_pool.tile([P, BLK], BF16)
            nc.vector.memset(wl, 0.0)
            nc.vector.memset(wr, 0.0)
            wps = warm_psum.tile([P, BLK], F32)
            for i in range(20):
                nc.tensor.matmul(wps, wl, wr, start=True, stop=True)

        # ---------------- t-phase ----------------
        with tc.tile_pool(name="t_psum", bufs=2, space="PSUM") as t_psum:
            for nb in range(NB):
                sl = slice(nb * BLK, (nb + 1) * BLK)
                nc.vector.tensor_copy(out=qT8[:, :, sl], in_=qTb[nb])
                for c2c in range(CC):
                    tps = t_psum.tile([P, BLK], F32, tag="tp", name=f"t{nb}_{c2c}")
                    for i in range(CC // 2):
                        nc.tensor.matmul(
                            tps,
                            sbM8[:, 2 * i : 2 * i + 2, c2c * P : (c2c + 1) * P],
                            qT8[:, 2 * i : 2 * i + 2, sl],
                            start=(i == 0),
                            stop=(i == CC // 2 - 1),
                            perf_mode=DR,
                        )
                    nc.vector.tensor_copy(out=tT8[:, c2c, sl], in_=tps)

        # ---------------- steady ----------------
        with (
            tc.tile_pool(name="pt_pool", bufs=1) as pt_pool,
            tc.tile_pool(name="e8_pool", bufs=2) as e8_pool,
            tc.tile_pool(name="out_pool", bufs=2) as out_pool,
            tc.tile_pool(name="rec_pool", bufs=2) as rec_pool,
            tc.tile_pool(name="s_psum", bufs=3, space="PSUM") as s_psum,
            tc.tile_pool(name="o_psum", bufs=2, space="PSUM") as o_psum,
            tc.tile_pool(name="o2_psum", bufs=2, space="PSUM") as o2_psum,
        ):
            PT16 = pt_pool.tile([P, NN, BLK], BF16)
            E8s = [
                e8_pool.tile([P, NN, BLK], F8, tag="e8", name=f"e8_{par}")
                for par in range(2)
            ]

            def s_block(nb):
                E8 = E8s[nb % 2]
                sl = slice(nb * BLK, (nb + 1) * BLK)
                for mc in range(NN):
                    sp = s_psum.tile([P, BLK], F32, tag="sp", name=f"sp{nb}_{mc}")
                    for i in range(CC // 2):
                        nc.tensor.matmul(
                            sp,
                            kT8[:, 2 * i : 2 * i + 2, mc * P : (mc + 1) * P],
                            tT8[:, 2 * i : 2 * i + 2, sl],
                            start=(i == 0),
                            stop=(i == CC // 2 - 1),
                            perf_mode=DR,
                        )
                    nc.scalar.activation(
                        out=PT16[:, mc, :], in_=sp, func=EXP,
                        scale=SCALE16, bias=ln16,
                    )
                    nc.vector.tensor_scalar(
                        out=E8[:, mc, :], in0=PT16[:, mc, :],
                        scalar1=16.0, scalar2=None, op0=SUB,
                    )

            def colsum_block():
                # 16*colsum(v) into idle PV psum slots; identical rows
                cs1 = o_psum.tile([P, BLK], F32, tag="op1", name="cs1")
                cs2 = o2_psum.tile([P, C - BLK], F32, tag="op2", name="cs2")
                for mc in range(NN):
                    nc.tensor.matmul(
                        cs1, all16, vstage[mc][:, 0:BLK],
                        start=(mc == 0), stop=(mc == NN - 1),
                    )
                    nc.tensor.matmul(
                        cs2, all16, vstage[mc][:, BLK:C],
                        start=(mc == 0), stop=(mc == NN - 1),
                    )
                nc.vector.tensor_copy(out=csum[:, 0:BLK], in_=cs1)
                nc.vector.tensor_copy(out=csum[:, BLK:C], in_=cs2)

            def pv_block(nb):
                E8 = E8s[nb % 2]
                for ns in range(4):
                    op1 = o_psum.tile([P, BLK], F32, tag="op1", name=f"o1_{nb}_{ns}")
                    op2 = o2_psum.tile(
                        [P, VW - BLK], F32, tag="op2", name=f"o2_{nb}_{ns}"
                    )
                    # inject 16*colsum via identity matmul (resets psum)
                    nc.tensor.matmul(
                        op1, ident, csum[:, 0:BLK], start=True, stop=False,
                    )
                    nc.tensor.matmul(
                        op2, ident, csum[:, BLK:VW], start=True, stop=False,
                    )
                    nsl = slice(ns * P, (ns + 1) * P)
                    for i in range(NN // 2):
                        lhs = E8[:, 2 * i : 2 * i + 2, nsl]
                        last = i == NN // 2 - 1
                        nc.tensor.matmul(
                            op1, lhs, vb8[:, 2 * i : 2 * i + 2, 0:BLK],
                            start=False, stop=last,
                            perf_mode=DR,
                        )
                        nc.tensor.matmul(
                            op2, lhs, vb8[:, 2 * i : 2 * i + 2, BLK:VW],
                            start=False, stop=last,
                            perf_mode=DR,
                        )
                    rec = rec_pool.tile([P, 1], F32, tag="rec", name=f"rc{nb}_{ns}")
                    nc.vector.reciprocal(out=rec, in_=op2[:, C - BLK : C - BLK + 1])
                    o_t = out_pool.tile([P, C], BF16, tag="ot", name=f"ot{nb}_{ns}")
                    nc.scalar.activation(
                        out=o_t[:, 0:BLK], in_=op1, func=COPY, scale=rec
                    )
                    nc.scalar.activation(
                        out=o_t[:, BLK:C], in_=op2[:, 0 : C - BLK], func=COPY,
                        scale=rec,
                    )
                    row0 = nb * BLK + ns * P
                    eng = nc.sync if ns % 2 == 0 else nc.gpsimd
                    eng.dma_start(out=out[row0 : row0 + P, :], in_=o_t)

            s_block(0)
            colsum_block()
            s_block(1)
            pv_block(0)
            s_block(2)
            pv_block(1)
            s_block(3)
            pv_block(2)
            pv_block(3)

    nc.compile()
    return nc


_NC = None


def _get_nc():
    global _NC
    if _NC is None:
        _NC = build_kernel()
    return _NC


def kernel(q_x, k_x, v_x, Wq, Wk):
    import ml_dtypes
    from concourse.bass_utils import run_bass_kernel_spmd

    bf = ml_dtypes.bfloat16
    f8 = ml_dtypes.float8_e4m3
    q_x = np.ascontiguousarray(np.asarray(q_x, dtype=np.float32)).astype(bf)
    k_x = np.ascontiguousarray(np.asarray(k_x, dtype=np.float32)).astype(bf)
    v_x = np.ascontiguousarray(np.asarray(v_x, dtype=np.float32)).astype(bf)
    Wq = np.asarray(Wq, dtype=np.float32)
    Wk = np.asarray(Wk, dtype=np.float32)
    # weight folding: S = q_x (Wq^T Wk) k_x^T; x16 to center fp8 range
    Mw = np.ascontiguousarray(16.0 * (Wq.T @ Wk)).astype(f8)

    nc = _get_nc()
    in_maps = [
        {"q_x": q_x[i], "k_x": k_x[i], "v_x": v_x[i], "Mw": Mw}
        for i in range(B)
    ]
    res = run_bass_kernel_spmd(nc, in_maps, core_ids=list(range(B)))
    return np.stack(
        [res.results[i]["out"].astype(np.float32) for i in range(B)], axis=0
    )


# revision 13
# speedup vs baseline: 1.4770x; 1.4770x over previous
"""Cross-attention Trainium2 kernel (B=8, N=2048, C=768, head=1).

reference:
  q = q_x @ Wq.T ; k = k_x @ Wk.T
  S = (q @ k.T) / 768 ; P = softmax(S, -1) ; out = P @ v_x

Strategy (per core, data-parallel over batch), fp8 DoubleRow everywhere:
  M16 = 16 * Wq.T @ Wk  (host, fp8)
  qT/kT via XBAR DMA-transpose (bf16, per-block jobs) -> cast to fp8
  t16T[c2, n] = sum_c1 M16[c1,c2] * qT[c1,n]     (fp8 DR, psum f32 -> fp8)
  S16[m, n]  = sum_c2 kT[c2,m] * t16T[c2,n]      (fp8 DR)  == 16 * S_true
  PT16 = exp(S16/(768*16) + ln16) = 16*exp(a)    (ACT, bf16)
  E16  = PT16 - 16 = 16*(exp(a)-1)               (DVE, fp8; |E16| ~ 0.6)
  O[n, 0:784] = 16*colsum([v|1]) + sum_m E16[m,n] * vb8[m, 0:784]
    colsum injected via identity f32r matmul (start of psum group),
    E-part via fp8 DR. col 768 = 16*Z (softmax denominator * 16).
  out[n, c] = O[n, c] / O[n, 768]    (bf16 out, host upcasts)

Mean-subtraction is load-bearing: attention here is near-uniform
(a ~ N(0, 0.036)), so out ~= colmean(v) and coherent fp8 noise on P or v
does not average down (naive fp8 P@v measures 3.6e-2 rel err, over the
2e-2 gate). Quantizing only the deviation E = P-1 scales that noise by
~0.036: measured 3.2e-3 end to end in numpy.

Engine split: PE does t/S/PV DR matmuls + colsum (woven after S(0), when
the v tiles have landed behind the XBAR transpose storm). ACT does exp
and the final normalize. DVE does q/t casts, E16, reciprocals. GpSimd
does the k/v/csum casts so a late v DMA can never stall the E16 chain.
PE order S(nb+1) before PV(nb) keeps exp/E16 latency off the PE path.
"""

import sys

sys.path.insert(0, "/opt/trn_rl_repo")

from contextlib import ExitStack

import numpy as np

import concourse.bass as bass
import concourse.mybir as mybir
import concourse.tile as tile
from concourse import bacc
from concourse.masks import make_identity

F32 = mybir.dt.float32
F32R = mybir.dt.float32r
BF16 = mybir.dt.bfloat16
F8 = mybir.dt.float8e4
DR = mybir.MatmulPerfMode.DoubleRow

B = 8
N = 2048
C = 768
P = 128
CC = C // P          # 6 chunks of the channel dim
NN = N // P          # 16 chunks of the sequence dim
BLK = 512            # free-dim block (PSUM bank = 512 f32)
NB = N // BLK        # 4 sequence blocks
VW = C + 16          # padded v width: [v | 1 | 0*15], stride mult of 16
SCALE16 = 1.0 / float(C * 16)
LN16 = float(np.log(16.0))
EXP = mybir.ActivationFunctionType.Exp
COPY = mybir.ActivationFunctionType.Copy
SUB = mybir.AluOpType.subtract


def build_kernel():
    nc = bacc.Bacc("TRN2", target_bir_lowering=False, debug=False, num_devices=B)
    q_x = nc.declare_dram_parameter("q_x", [N, C], BF16, isOutput=False)
    k_x = nc.declare_dram_parameter("k_x", [N, C], BF16, isOutput=False)
    v_x = nc.declare_dram_parameter("v_x", [N, C], BF16, isOutput=False)
    Mw = nc.declare_dram_parameter("Mw", [C, C], F8, isOutput=False)
    out = nc.declare_dram_parameter("out", [N, C], BF16, isOutput=True)

    with tile.TileContext(nc) as tc, ExitStack() as ctx:
        persist = ctx.enter_context(tc.tile_pool(name="persist", bufs=1))
        qT8 = persist.tile([P, CC, N], F8)      # q_x.T fp8 [c1, n]
        kT8 = persist.tile([P, CC, N], F8)      # k_x.T fp8 [c2, m]
        tT8 = persist.tile([P, CC, N], F8)      # t16.T fp8 [c2, n]
        sbM8 = persist.tile([P, CC, C], F8)     # M16 [c1, c2]
        vb8 = persist.tile([P, NN, VW], F8)     # [v | 1 | 0...] fp8
        csum = persist.tile([P, VW], F32R)      # 16*colsum [v | 1 | 0], all rows
        identf = persist.tile([P, P], F32)
        ident = persist.tile([P, P], F32R)
        all16 = persist.tile([P, P], BF16)
        ln16 = persist.tile([P, 1], F32)
        onepad = persist.tile([P, NN, VW - C], F32)
        cspad = persist.tile([P, VW - C], F32)
        make_identity(nc, identf)
        nc.vector.tensor_copy(out=ident, in_=identf)
        nc.vector.memset(all16, 16.0)
        nc.vector.memset(ln16, LN16)
        nc.vector.memset(onepad[:, :, 0:1], 1.0)
        nc.vector.memset(onepad[:, :, 1:], 0.0)
        nc.vector.tensor_copy(out=vb8[:, :, C:VW], in_=onepad)
        nc.vector.memset(cspad[:, 0:1], 32768.0)
        nc.vector.memset(cspad[:, 1:], 0.0)
        nc.vector.tensor_copy(out=csum[:, C:VW], in_=cspad)

        bfpool = ctx.enter_context(tc.tile_pool(name="bfstage", bufs=1))
        qTb = [bfpool.tile([P, CC, BLK], BF16, name=f"qTb{nb}") for nb in range(NB)]
        kTb = [bfpool.tile([P, CC, BLK], BF16, name=f"kTb{nb}") for nb in range(NB)]
        vstage = [bfpool.tile([P, C], BF16, name=f"v{mc}") for mc in range(NN)]

        # ---------------- prologue DMAs ----------------
        # sync ring: M16 chunks then q transposes (t-matmul critical path);
        # scalar ring: k transposes (needed from S(0)); gpsimd ring: v tiles.
        for c1c in range(CC):
            nc.sync.dma_start(
                out=sbM8[:, c1c, :], in_=Mw[c1c * P : (c1c + 1) * P, :]
            )
        for nb in range(NB):
            nc.sync.dma_start_transpose(
                out=qTb[nb], in_=q_x[nb * BLK : (nb + 1) * BLK, :]
            )
            nc.scalar.dma_start_transpose(
                out=kTb[nb], in_=k_x[nb * BLK : (nb + 1) * BLK, :]
            )
        for mc in range(NN):
            nc.gpsimd.dma_start(out=vstage[mc], in_=v_x[mc * P : (mc + 1) * P, :])

        # ---------------- PE warmup (HAM un-throttle) ----------------
        with (
            tc.tile_pool(name="warm", bufs=1) as warm_pool,
            tc.tile_pool(name="warm_psum", bufs=1, space="PSUM") as warm_psum,
        ):
            wl = warm_pool.tile([P, P], BF16)
            wr = warm_pool.tile([P, BLK], BF16)
            nc.vector.memset(wl, 0.0)
            nc.vector.memset(wr, 0.0)
            wps = warm_psum.tile([P, BLK], F32)
            for i in range(20):
                nc.tensor.matmul(wps, wl, wr, start=True, stop=True)

        # ---------------- t-phase ----------------
        with tc.tile_pool(name="t_psum", bufs=2, space="PSUM") as t_psum:
            for nb in range(NB):
                sl = slice(nb * BLK, (nb + 1) * BLK)
                nc.vector.tensor_copy(out=qT8[:, :, sl], in_=qTb[nb])
                for c2c in range(CC):
                    tps = t_psum.tile([P, BLK], F32, tag="tp", name=f"t{nb}_{c2c}")
                    for i in range(CC // 2):
                        nc.tensor.matmul(
                            tps,
                            sbM8[:, 2 * i : 2 * i + 2, c2c * P : (c2c + 1) * P],
                            qT8[:, 2 * i : 2 * i + 2, sl],
                            start=(i == 0),
                            stop=(i == CC // 2 - 1),
                            perf_mode=DR,
                        )
                    nc.vector.tensor_copy(out=tT8[:, c2c, sl], in_=tps)
            for nb in range(NB):
                nc.vector.tensor_copy(
                    out=kT8[:, :, nb * BLK : (nb + 1) * BLK], in_=kTb[nb]
                )
            for mc in range(NN):
                nc.vector.tensor_copy(out=vb8[:, mc, 0:C], in_=vstage[mc])

        # ---------------- steady ----------------
        with (
            tc.tile_pool(name="pt_pool", bufs=1) as pt_pool,
            tc.tile_pool(name="e8_pool", bufs=2) as e8_pool,
            tc.tile_pool(name="out_pool", bufs=2) as out_pool,
            tc.tile_pool(name="rec_pool", bufs=2) as rec_pool,
            tc.tile_pool(name="s_psum", bufs=3, space="PSUM") as s_psum,
            tc.tile_pool(name="o_psum", bufs=2, space="PSUM") as o_psum,
            tc.tile_pool(name="o2_psum", bufs=2, space="PSUM") as o2_psum,
        ):
            PT16 = pt_pool.tile([P, NN, BLK], BF16)
            E8s = [
                e8_pool.tile([P, NN, BLK], F8, tag="e8", name=f"e8_{par}")
                for par in range(2)
            ]

            def s_block(nb):
                E8 = E8s[nb % 2]
                sl = slice(nb * BLK, (nb + 1) * BLK)
                for mc in range(NN):
                    sp = s_psum.tile([P, BLK], F32, tag="sp", name=f"sp{nb}_{mc}")
                    for i in range(CC // 2):
                        nc.tensor.matmul(
                            sp,
                            kT8[:, 2 * i : 2 * i + 2, mc * P : (mc + 1) * P],
                            tT8[:, 2 * i : 2 * i + 2, sl],
                            start=(i == 0),
                            stop=(i == CC // 2 - 1),
                            perf_mode=DR,
                        )
                    nc.scalar.activation(
                        out=PT16[:, mc, :], in_=sp, func=EXP,
                        scale=SCALE16, bias=ln16,
                    )
                    nc.vector.tensor_scalar(
                        out=E8[:, mc, :], in0=PT16[:, mc, :],
                        scalar1=16.0, scalar2=None, op0=SUB,
                    )

            def colsum_block():
                # 16*colsum(v) into idle PV psum slots; identical rows
                cs1 = o_psum.tile([P, BLK], F32, tag="op1", name="cs1")
                cs2 = o2_psum.tile([P, C - BLK], F32, tag="op2", name="cs2")
                for mc in range(NN):
                    nc.tensor.matmul(
                        cs1, all16, vstage[mc][:, 0:BLK],
                        start=(mc == 0), stop=(mc == NN - 1),
                    )
                    nc.tensor.matmul(
                        cs2, all16, vstage[mc][:, BLK:C],
                        start=(mc == 0), stop=(mc == NN - 1),
                    )
                nc.scalar.activation(out=csum[:, 0:BLK], in_=cs1, func=COPY)
                nc.scalar.activation(out=csum[:, BLK:C], in_=cs2, func=COPY)

            def pv_block(nb):
                E8 = E8s[nb % 2]
                for ns in range(4):
                    op1 = o_psum.tile([P, BLK], F32, tag="op1", name=f"o1_{nb}_{ns}")
                    op2 = o2_psum.tile(
                        [P, VW - BLK], F32, tag="op2", name=f"o2_{nb}_{ns}"
                    )
                    # inject 16*colsum via identity matmul (resets psum)
                    nc.tensor.matmul(
                        op1, ident, csum[:, 0:BLK], start=True, stop=False,
                    )
                    nc.tensor.matmul(
                        op2, ident, csum[:, BLK:VW], start=True, stop=False,
                    )
                    nsl = slice(ns * P, (ns + 1) * P)
                    for i in range(NN // 2):
                        lhs = E8[:, 2 * i : 2 * i + 2, nsl]
                        last = i == NN // 2 - 1
                        nc.tensor.matmul(
                            op1, lhs, vb8[:, 2 * i : 2 * i + 2, 0:BLK],
                            start=False, stop=last,
                            perf_mode=DR,
                        )
                        nc.tensor.matmul(
                            op2, lhs, vb8[:, 2 * i : 2 * i + 2, BLK:VW],
                            start=False, stop=last,
                            perf_mode=DR,
                        )
                    rec = rec_pool.tile([P, 1], F32, tag="rec", name=f"rc{nb}_{ns}")
                    nc.vector.reciprocal(out=rec, in_=op2[:, C - BLK : C - BLK + 1])
                    o_t = out_pool.tile([P, C], BF16, tag="ot", name=f"ot{nb}_{ns}")
                    nc.scalar.activation(
                        out=o_t[:, 0:BLK], in_=op1, func=COPY, scale=rec
                    )
                    nc.scalar.activation(
                        out=o_t[:, BLK:C], in_=op2[:, 0 : C - BLK], func=COPY,
                        scale=rec,
                    )
                    row0 = nb * BLK + ns * P
                    nc.sync.dma_start(out=out[row0 : row0 + P, :], in_=o_t)

            colsum_block()
            s_block(0)
            s_block(1)
            pv_block(0)
            s_block(2)
            pv_block(1)
            s_block(3)
            pv_block(2)
            pv_block(3)

    nc.compile()
    return nc


_NC = None


def _get_nc():
    global _NC
    if _NC is None:
        _NC = build_kernel()
    return _NC


def kernel(q_x, k_x, v_x, Wq, Wk):
    import ml_dtypes
    from concourse.bass_utils import run_bass_kernel_spmd

    bf = ml_dtypes.bfloat16
    f8 = ml_dtypes.float8_e4m3
    q_x = np.ascontiguousarray(np.asarray(q_x, dtype=np.float32)).astype(bf)
    k_x = np.ascontiguousarray(np.asarray(k_x, dtype=np.float32)).astype(bf)
    v_x = np.ascontiguousarray(np.asarray(v_x, dtype=np.float32)).astype(bf)
    Wq = np.asarray(Wq, dtype=np.float32)
    Wk = np.asarray(Wk, dtype=np.float32)
    # weight folding: S = q_x (Wq^T Wk) k_x^T; x16 to center fp8 range
    Mw = np.ascontiguousarray(16.0 * (Wq.T @ Wk)).astype(f8)

    nc = _get_nc()
    in_maps = [
        {"q_x": q_x[i], "k_x": k_x[i], "v_x": v_x[i], "Mw": Mw}
        for i in range(B)
    ]
    res = run_bass_kernel_spmd(nc, in_maps, core_ids=list(range(B)))
    return np.stack(
        [res.results[i]["out"].astype(np.float32) for i in range(B)], axis=0
    )


# revision 14
# speedup vs baseline: 1.4794x; 1.0016x over previous
"""Cross-attention Trainium2 kernel (B=8, N=2048, C=768, head=1).

reference:
  q = q_x @ Wq.T ; k = k_x @ Wk.T
  S = (q @ k.T) / 768 ; P = softmax(S, -1) ; out = P @ v_x

Strategy (per core, data-parallel over batch), fp8 DoubleRow everywhere:
  M16 = 16 * Wq.T @ Wk  (host, fp8)
  qT/kT via XBAR DMA-transpose (bf16, per-block jobs) -> cast to fp8
  t16T[c2, n] = sum_c1 M16[c1,c2] * qT[c1,n]     (fp8 DR, psum f32 -> fp8)
  S16[m, n]  = sum_c2 kT[c2,m] * t16T[c2,n]      (fp8 DR)  == 16 * S_true
  PT16 = exp(S16/(768*16) + ln16) = 16*exp(a)    (ACT, bf16)
  E16  = PT16 - 16 = 16*(exp(a)-1)               (DVE, fp8; |E16| ~ 0.6)
  O[n, 0:784] = 16*colsum([v|1]) + sum_m E16[m,n] * vb8[m, 0:784]
    colsum injected via identity f32r matmul (start of psum group),
    E-part via fp8 DR. col 768 = 16*Z (softmax denominator * 16).
  out[n, c] = O[n, c] / O[n, 768]    (bf16 out, host upcasts)

Mean-subtraction is load-bearing: attention here is near-uniform
(a ~ N(0, 0.036)), so out ~= colmean(v) and coherent fp8 noise on P or v
does not average down (naive fp8 P@v measures 3.6e-2 rel err, over the
2e-2 gate). Quantizing only the deviation E = P-1 scales that noise by
~0.036: measured 3.2e-3 end to end in numpy.

Engine split: PE does t/S/PV DR matmuls + colsum (woven after S(0), when
the v tiles have landed behind the XBAR transpose storm). ACT does exp
and the final normalize. DVE does q/t casts, E16, reciprocals. GpSimd
does the k/v/csum casts so a late v DMA can never stall the E16 chain.
PE order S(nb+1) before PV(nb) keeps exp/E16 latency off the PE path.
"""

import sys

sys.path.insert(0, "/opt/trn_rl_repo")

from contextlib import ExitStack

import numpy as np

import concourse.bass as bass
import concourse.mybir as mybir
import concourse.tile as tile
from concourse import bacc
from concourse.masks import make_identity

F32 = mybir.dt.float32
F32R = mybir.dt.float32r
BF16 = mybir.dt.bfloat16
F8 = mybir.dt.float8e4
DR = mybir.MatmulPerfMode.DoubleRow

B = 8
N = 2048
C = 768
P = 128
CC = C // P          # 6 chunks of the channel dim
NN = N // P          # 16 chunks of the sequence dim
BLK = 512            # free-dim block (PSUM bank = 512 f32)
NB = N // BLK        # 4 sequence blocks
VW = C + 16          # padded v width: [v | 1 | 0*15], stride mult of 16
SCALE16 = 1.0 / float(C * 16)
LN16 = float(np.log(16.0))
EXP = mybir.ActivationFunctionType.Exp
COPY = mybir.ActivationFunctionType.Copy
SUB = mybir.AluOpType.subtract


def build_kernel():
    nc = bacc.Bacc("TRN2", target_bir_lowering=False, debug=False, num_devices=B)
    q_x = nc.declare_dram_parameter("q_x", [N, C], BF16, isOutput=False)
    k_x = nc.declare_dram_parameter("k_x", [N, C], BF16, isOutput=False)
    v_x = nc.declare_dram_parameter("v_x", [N, C], BF16, isOutput=False)
    Mw = nc.declare_dram_parameter("Mw", [C, C], F8, isOutput=False)
    out = nc.declare_dram_parameter("out", [N, C], BF16, isOutput=True)

    with tile.TileContext(nc) as tc, ExitStack() as ctx:
        persist = ctx.enter_context(tc.tile_pool(name="persist", bufs=1))
        qT8 = persist.tile([P, CC, N], F8)      # q_x.T fp8 [c1, n]
        kT8 = persist.tile([P, CC, N], F8)      # k_x.T fp8 [c2, m]
        tT8 = persist.tile([P, CC, N], F8)      # t16.T fp8 [c2, n]
        sbM8 = persist.tile([P, CC, C], F8)     # M16 [c1, c2]
        vb8 = persist.tile([P, NN, VW], F8)     # [v | 1 | 0...] fp8
        csum = persist.tile([P, VW], F32R)      # 16*colsum [v | 1 | 0], all rows
        identf = persist.tile([P, P], F32)
        ident = persist.tile([P, P], F32R)
        all16 = persist.tile([P, P], BF16)
        ln16 = persist.tile([P, 1], F32)
        onepad = persist.tile([P, NN, VW - C], F32)
        cspad = persist.tile([P, VW - C], F32)
        make_identity(nc, identf)
        nc.vector.tensor_copy(out=ident, in_=identf)
        nc.vector.memset(all16, 16.0)
        nc.vector.memset(ln16, LN16)
        nc.vector.memset(onepad[:, :, 0:1], 1.0)
        nc.vector.memset(onepad[:, :, 1:], 0.0)
        nc.vector.tensor_copy(out=vb8[:, :, C:VW], in_=onepad)
        nc.vector.memset(cspad[:, 0:1], 32768.0)
        nc.vector.memset(cspad[:, 1:], 0.0)
        nc.vector.tensor_copy(out=csum[:, C:VW], in_=cspad)

        bfpool = ctx.enter_context(tc.tile_pool(name="bfstage", bufs=1))
        qTb = [bfpool.tile([P, CC, BLK], BF16, name=f"qTb{nb}") for nb in range(NB)]
        kTb = [bfpool.tile([P, CC, BLK], BF16, name=f"kTb{nb}") for nb in range(NB)]
        vstage = [bfpool.tile([P, C], BF16, name=f"v{mc}") for mc in range(NN)]

        # ---------------- prologue DMAs ----------------
        # Two hwdge rings share the 16 DMA engines; a transpose job occupies
        # all of them for ~3us. Issue order = need order: q blocks (t-phase)
        # split across both rings, then k blocks (S-phase), then v (colsum).
        for c1c in range(CC):
            nc.sync.dma_start(
                out=sbM8[:, c1c, :], in_=Mw[c1c * P : (c1c + 1) * P, :]
            )
        for nb in range(NB):
            ring = nc.sync if nb % 2 == 0 else nc.scalar
            ring.dma_start_transpose(
                out=qTb[nb], in_=q_x[nb * BLK : (nb + 1) * BLK, :]
            )
        for nb in range(NB):
            ring = nc.sync if nb % 2 == 0 else nc.scalar
            ring.dma_start_transpose(
                out=kTb[nb], in_=k_x[nb * BLK : (nb + 1) * BLK, :]
            )
        for mc in range(NN):
            ring = nc.sync if mc % 2 == 0 else nc.scalar
            ring.dma_start(out=vstage[mc], in_=v_x[mc * P : (mc + 1) * P, :])

        # ---------------- PE warmup (HAM un-throttle) ----------------
        with (
            tc.tile_pool(name="warm", bufs=1) as warm_pool,
            tc.tile_pool(name="warm_psum", bufs=1, space="PSUM") as warm_psum,
        ):
            wl = warm_pool.tile([P, P], BF16)
            wr = warm_pool.tile([P, BLK], BF16)
            nc.vector.memset(wl, 0.0)
            nc.vector.memset(wr, 0.0)
            wps = warm_psum.tile([P, BLK], F32)
            for i in range(20):
                nc.tensor.matmul(wps, wl, wr, start=True, stop=True)

        # ---------------- t-phase ----------------
        with tc.tile_pool(name="t_psum", bufs=2, space="PSUM") as t_psum:
            for nb in range(NB):
                sl = slice(nb * BLK, (nb + 1) * BLK)
                nc.vector.tensor_copy(out=qT8[:, :, sl], in_=qTb[nb])
                for c2c in range(CC):
                    tps = t_psum.tile([P, BLK], F32, tag="tp", name=f"t{nb}_{c2c}")
                    for i in range(CC // 2):
                        nc.tensor.matmul(
                            tps,
                            sbM8[:, 2 * i : 2 * i + 2, c2c * P : (c2c + 1) * P],
                            qT8[:, 2 * i : 2 * i + 2, sl],
                            start=(i == 0),
                            stop=(i == CC // 2 - 1),
                            perf_mode=DR,
                        )
                    nc.vector.tensor_copy(out=tT8[:, c2c, sl], in_=tps)
            for nb in range(NB):
                nc.vector.tensor_copy(
                    out=kT8[:, :, nb * BLK : (nb + 1) * BLK], in_=kTb[nb]
                )
            for mc in range(NN):
                nc.vector.tensor_copy(out=vb8[:, mc, 0:C], in_=vstage[mc])

        # ---------------- steady ----------------
        with (
            tc.tile_pool(name="pt_pool", bufs=1) as pt_pool,
            tc.tile_pool(name="e8_pool", bufs=2) as e8_pool,
            tc.tile_pool(name="out_pool", bufs=2) as out_pool,
            tc.tile_pool(name="rec_pool", bufs=2) as rec_pool,
            tc.tile_pool(name="s_psum", bufs=3, space="PSUM") as s_psum,
            tc.tile_pool(name="o_psum", bufs=2, space="PSUM") as o_psum,
            tc.tile_pool(name="o2_psum", bufs=2, space="PSUM") as o2_psum,
        ):
            PT16 = pt_pool.tile([P, NN, BLK], BF16)
            E8s = [
                e8_pool.tile([P, NN, BLK], F8, tag="e8", name=f"e8_{par}")
                for par in range(2)
            ]

            def s_block(nb):
                E8 = E8s[nb % 2]
                sl = slice(nb * BLK, (nb + 1) * BLK)
                for mc in range(NN):
                    sp = s_psum.tile([P, BLK], F32, tag="sp", name=f"sp{nb}_{mc}")
                    for i in range(CC // 2):
                        nc.tensor.matmul(
                            sp,
                            kT8[:, 2 * i : 2 * i + 2, mc * P : (mc + 1) * P],
                            tT8[:, 2 * i : 2 * i + 2, sl],
                            start=(i == 0),
                            stop=(i == CC // 2 - 1),
                            perf_mode=DR,
                        )
                    nc.scalar.activation(
                        out=PT16[:, mc, :], in_=sp, func=EXP,
                        scale=SCALE16, bias=ln16,
                    )
                    nc.vector.tensor_scalar(
                        out=E8[:, mc, :], in0=PT16[:, mc, :],
                        scalar1=16.0, scalar2=None, op0=SUB,
                    )

            def colsum_block():
                # 16*colsum(v) into idle PV psum slots; identical rows
                cs1 = o_psum.tile([P, BLK], F32, tag="op1", name="cs1")
                cs2 = o2_psum.tile([P, C - BLK], F32, tag="op2", name="cs2")
                for mc in range(NN):
                    nc.tensor.matmul(
                        cs1, all16, vstage[mc][:, 0:BLK],
                        start=(mc == 0), stop=(mc == NN - 1),
                    )
                    nc.tensor.matmul(
                        cs2, all16, vstage[mc][:, BLK:C],
                        start=(mc == 0), stop=(mc == NN - 1),
                    )
                nc.scalar.activation(out=csum[:, 0:BLK], in_=cs1, func=COPY)
                nc.scalar.activation(out=csum[:, BLK:C], in_=cs2, func=COPY)

            def pv_block(nb):
                E8 = E8s[nb % 2]
                for ns in range(4):
                    op1 = o_psum.tile([P, BLK], F32, tag="op1", name=f"o1_{nb}_{ns}")
                    op2 = o2_psum.tile(
                        [P, VW - BLK], F32, tag="op2", name=f"o2_{nb}_{ns}"
                    )
                    # inject 16*colsum via identity matmul (resets psum)
                    nc.tensor.matmul(
                        op1, ident, csum[:, 0:BLK], start=True, stop=False,
                    )
                    nc.tensor.matmul(
                        op2, ident, csum[:, BLK:VW], start=True, stop=False,
                    )
                    nsl = slice(ns * P, (ns + 1) * P)
                    for i in range(NN // 2):
                        lhs = E8[:, 2 * i : 2 * i + 2, nsl]
                        last = i == NN // 2 - 1
                        nc.tensor.matmul(
                            op1, lhs, vb8[:, 2 * i : 2 * i + 2, 0:BLK],
                            start=False, stop=last,
                            perf_mode=DR,
                        )
                        nc.tensor.matmul(
                            op2, lhs, vb8[:, 2 * i : 2 * i + 2, BLK:VW],
                            start=False, stop=last,
                            perf_mode=DR,
                        )
                    rec = rec_pool.tile([P, 1], F32, tag="rec", name=f"rc{nb}_{ns}")
                    nc.vector.reciprocal(out=rec, in_=op2[:, C - BLK : C - BLK + 1])
                    o_t = out_pool.tile([P, C], BF16, tag="ot", name=f"ot{nb}_{ns}")
                    nc.scalar.activation(
                        out=o_t[:, 0:BLK], in_=op1, func=COPY, scale=rec
                    )
                    nc.scalar.activation(
                        out=o_t[:, BLK:C], in_=op2[:, 0 : C - BLK], func=COPY,
                        scale=rec,
                    )
                    row0 = nb * BLK + ns * P
                    if nb == NB - 1:
                        nc.sync.dma_start(
                            out=out[row0 : row0 + P, 0 : C // 2],
                            in_=o_t[:, 0 : C // 2],
                        )
                        nc.scalar.dma_start(
                            out=out[row0 : row0 + P, C // 2 : C],
                            in_=o_t[:, C // 2 : C],
                        )
                    else:
                        ring = nc.sync if ns % 2 == 0 else nc.scalar
                        ring.dma_start(out=out[row0 : row0 + P, :], in_=o_t)

            colsum_block()
            s_block(0)
            s_block(1)
            pv_block(0)
            s_block(2)
            pv_block(1)
            s_block(3)
            pv_block(2)
            pv_block(3)

    nc.compile()
    return nc


_NC = None


def _get_nc():
    global _NC
    if _NC is None:
        _NC = build_kernel()
    return _NC


def kernel(q_x, k_x, v_x, Wq, Wk):
    import ml_dtypes
    from concourse.bass_utils import run_bass_kernel_spmd

    bf = ml_dtypes.bfloat16
    f8 = ml_dtypes.float8_e4m3
    q_x = np.ascontiguousarray(np.asarray(q_x, dtype=np.float32)).astype(bf)
    k_x = np.ascontiguousarray(np.asarray(k_x, dtype=np.float32)).astype(bf)
    v_x = np.ascontiguousarray(np.asarray(v_x, dtype=np.float32)).astype(bf)
    Wq = np.asarray(Wq, dtype=np.float32)
    Wk = np.asarray(Wk, dtype=np.float32)
    # weight folding: S = q_x (Wq^T Wk) k_x^T; x16 to center fp8 range
    Mw = np.ascontiguousarray(16.0 * (Wq.T @ Wk)).astype(f8)

    nc = _get_nc()
    in_maps = [
        {"q_x": q_x[i], "k_x": k_x[i], "v_x": v_x[i], "Mw": Mw}
        for i in range(B)
    ]
    res = run_bass_kernel_spmd(nc, in_maps, core_ids=list(range(B)))
    return np.stack(
        [res.results[i]["out"].astype(np.float32) for i in range(B)], axis=0
    )
